# revision 2
# baseline (speedup 1.0000x reference)
"""Trainium2 Bass kernel: multi-head attention (B=2, S=2048, D=1024, H=16,
d_k=64) with RoPE and masked softmax, sharded over 8 NeuronCores as
(batch x head-group): core = b*4 + g handles batch b, heads [4g, 4g+4).

Per-core device program (all matmul inputs bf16, PSUM accumulation f32):
  1. All x inputs resident in SBUF (24 row-major bf16 tiles, two HWDGE
     queues + Pool SWDGE); projections run k-outer into 2-chunk PSUM
     tiles so accumulation streams with the input DMAs.
  2. RoPE rotate-half via a PE permutation matmul (sign folded into a
     128x128 block-diag P), then dst = stage*cos + rot*sin on DVE/Pool.
  3. Causal shrink: scores/exp/PV touch only [j0:] per k-tile; the
     diagonal-block mask is a 0/1 multiply on e_t after exp (SBUF, Pool)
     -- scores stay O(10) so exp never overflows. A ones column per
     65-wide V head block makes the PV matmul emit softmax denominators.
  4. Phase interleave: [Qt0 Kt0][V 0..7][attn qc0 h0 h1][Qt1 Kt1]
     [attn qc0 h2 h3 + V 8..15][oproj qc0 inside attn qc1][attn qc1]
     [oproj qc1]; the Activation engine (exp, the phase-2 pacer) starts
     ~25us in. Head normalization (reciprocal -> K=1 outer-product
     broadcast -> scale) is emitted deferred into the next head's score
     stream so the in-order PE never parks on the DVE reciprocal.
  5. Single PSUM pool: sc 3x[128,1024] + ctx 1x[128,1024] = all 8 banks,
     shared by projection/rot/V/scores/broadcast/output-proj psums.
Output partials are written bf16; the host sums the 4 group partials per
batch in f32. Wait counts >1 are hoisted onto single-wait no-ops after
scheduling (walrus codegen limitation).
"""
import sys

sys.path.insert(0, "/opt/trn_rl_repo")

from contextlib import ExitStack

import numpy as np

import concourse.bass as bass
import concourse.mybir as mybir
import concourse.tile as tile

FP = mybir.dt.float32
FPR = mybir.dt.float32r
BF = mybir.dt.bfloat16
EXP = mybir.ActivationFunctionType.Exp

D = 1024        # d_model
S = 2048        # sequence length
NB = 2          # batches
HPG = 4         # heads per group (= per core)
DK = 64         # head dim
F = HPG * DK    # 256 = group feature width
KT = D // 128   # 8 contraction tiles for projections
ST = S // 128   # 16 seq tiles
QCW = 1024      # q-chunk width (= 2 PSUM banks)
NQC = S // QCW  # 2
NEG = -1e9

_nc_cache = {}


def _mm(nc, out, lhsT, rhs, **kw):
    nc.tensor.matmul(out, lhsT, rhs, **kw)


def _hoist_waits(nc):
    """Several walrus codegen structs (fused-LDW matmul, pseudo direct2d
    DMA, ...) only have room for a single sync wait. Hoist every limited
    instruction's waits (when >1) onto same-engine no-ops inserted just
    before it."""
    f = nc.m.functions[0]

    def engine_builder(eng):
        return {
            mybir.EngineType.PE: nc.tensor,
            mybir.EngineType.DVE: nc.vector,
            mybir.EngineType.Activation: nc.scalar,
            mybir.EngineType.Pool: nc.gpsimd,
            mybir.EngineType.SP: nc.sync,
        }[eng]

    def fresh_nop(eng):
        inst = engine_builder(eng).nop().ins
        for b in f.blocks:
            for i, x in enumerate(b.instructions):
                if x is inst:
                    del b.instructions[i]
                    return inst
        raise RuntimeError("created nop not found in any block")

    total = 0
    for blk in f.blocks:
        out = []
        for inst in blk.instructions:
            si = inst.sync_info
            if si is not None and len(si.on_wait) > 1:
                for w in si.on_wait[:-1]:
                    nop = fresh_nop(inst.engine)
                    nop.sync_info = mybir.SyncInfo(on_wait=[w], on_update=[])
                    out.append(nop)
                    total += 1
                inst.sync_info = mybir.SyncInfo(on_wait=[si.on_wait[-1]],
                                                on_update=list(si.on_update))
            out.append(inst)
        blk.instructions[:] = out
    return total


def build_nc(mask_mode):
    """mask_mode: 'causal' | 'full' | 'general'."""
    assert mask_mode in ("causal", "full", "general")
    nc = bass.Bass("TRN2", target_bir_lowering=False, debug=False, num_devices=8)

    xqT = nc.dram_tensor("xqT", [D, S], BF, kind="ExternalInput").ap()
    xkT = nc.dram_tensor("xkT", [D, S], BF, kind="ExternalInput").ap()
    xvT = nc.dram_tensor("xvT", [D, S], BF, kind="ExternalInput").ap()
    wqT = nc.dram_tensor("wqT", [D, F], BF, kind="ExternalInput").ap()
    wkT = nc.dram_tensor("wkT", [D, F], BF, kind="ExternalInput").ap()
    wvT = nc.dram_tensor("wvT", [D, F], BF, kind="ExternalInput").ap()
    woT = nc.dram_tensor("woT", [F, D], BF, kind="ExternalInput").ap()
    cosd = nc.dram_tensor("cosS", [128, S], BF, kind="ExternalInput").ap()
    sind = nc.dram_tensor("sinS", [128, S], BF, kind="ExternalInput").ap()
    protd = nc.dram_tensor("protD", [128, 128], BF, kind="ExternalInput").ap()
    if mask_mode == "general":
        biasT = nc.dram_tensor("biasT", [S, S], BF, kind="ExternalInput").ap()
    if mask_mode == "causal":
        # 0/1 band: trimulD[k, q] = 1 if k <= q else 0 (within the diag block)
        trimulD = nc.dram_tensor("trimulD", [128, 128], BF, kind="ExternalInput").ap()
    outp = nc.dram_tensor("outp", [S, D], BF, kind="ExternalOutput").ap()

    causal = mask_mode == "causal"

    with tile.TileContext(nc) as tc, ExitStack() as ctx:
        const = ctx.enter_context(tc.tile_pool(name="const", bufs=1))
        qk = ctx.enter_context(tc.tile_pool(name="qk", bufs=1))
        xres = ctx.enter_context(tc.tile_pool(name="xres", bufs=8))
        rpool = ctx.enter_context(tc.tile_pool(name="rope", bufs=4))
        psum = ctx.enter_context(tc.tile_pool(name="psum", bufs=1, space="PSUM"))
        epool = ctx.enter_context(tc.tile_pool(name="exp", bufs=6))
        npool = ctx.enter_context(tc.tile_pool(name="norm", bufs=2))
        opool = ctx.enter_context(tc.tile_pool(name="ost", bufs=6))
        if mask_mode == "general":
            bpool = ctx.enter_context(tc.tile_pool(name="bias", bufs=2))

        wq_sb = const.tile([128, KT * F], BF)
        wk_sb = const.tile([128, KT * F], BF)
        wv_sb = const.tile([128, KT * F], BF)
        wo_sb = const.tile([128, 2 * D], BF)
        cos_sb = const.tile([128, S], BF)
        sin_sb = const.tile([128, S], BF)
        prot_sb = const.tile([128, 128], BF)
        ones_f32 = const.tile([1, 64], FP)
        nc.vector.memset(ones_f32[:], 1.0)
        ones_col = const.tile([1, 64], FPR)
        nc.vector.tensor_copy(ones_col[:], ones_f32[:])
        ones64 = const.tile([128, 64], BF)
        nc.vector.memset(ones64[:], 1.0)

        # --- DMA issue: everything up-front on both queues, need-ordered ---
        # SP queue:  wq, xq even, wk, xk even, wv, xv even, wo
        # Act queue: cos/sin/prot/trimul, xq odd, xk odd, xv odd
        wq_r = wq_sb[:].rearrange("p (k f) -> p k f", k=KT)
        wqT_r = wqT[:].rearrange("(k p) f -> p k f", p=128)
        nc.sync.dma_start(wq_r[:, 0:4], wqT_r[:, 0:4])
        nc.scalar.dma_start(wq_r[:, 4:KT], wqT_r[:, 4:KT])

        def load_x(x_d, label, odd_eng):
            tiles = []
            for k in range(KT):
                xt = xres.tile([128, S], BF, tag=f"{label}", name=f"{label}{k}")
                dma_eng = nc.sync if k % 2 == 0 else odd_eng
                dma_eng.dma_start(xt[:], x_d[k * 128:(k + 1) * 128, :])
                tiles.append(xt)
            return tiles

        xq_t = load_x(xqT, "xq", nc.scalar)
        nc.scalar.dma_start(cos_sb[:], cosd[:])
        nc.scalar.dma_start(sin_sb[:], sind[:])
        nc.scalar.dma_start(prot_sb[:], protd[:])
        if causal:
            trimul_sb = const.tile([128, 128], BF)
            nc.scalar.dma_start(trimul_sb[:], trimulD[:])
        nc.gpsimd.dma_start(wk_sb[:].rearrange("p (k f) -> p k f", k=KT),
                            wkT[:].rearrange("(k p) f -> p k f", p=128))
        xk_t = load_x(xkT, "xk", nc.gpsimd)
        nc.sync.dma_start(wv_sb[:].rearrange("p (k f) -> p k f", k=KT),
                          wvT[:].rearrange("(k p) f -> p k f", p=128))
        xv_t = load_x(xvT, "xv", nc.gpsimd)
        nc.sync.dma_start(wo_sb[:].rearrange("p (t e) -> p t e", t=2),
                          woT[:].rearrange("(t p) e -> p t e", p=128))

        # persistent activations: [p, t*S + s] layouts (t-tile 0: heads 0,1;
        # t-tile 1: heads 2,3 of the group)
        qt_sb = qk.tile([128, 2 * S], BF)
        kt_sb = qk.tile([128, 2 * S], BF)
        # V in [s, f] layout with a ones column per head: 65-wide head blocks
        v_sb = qk.tile([128, ST * HPG * 65], BF)
        ctxn_sb = qk.tile([128, 2 * S], BF)

        ones_ap = v_sb[:].rearrange("p (b c) -> p b c", c=65)[:, :, 64:65]
        nc.vector.tensor_copy(ones_ap, ones64[:].rearrange("p (b o) -> p b o", o=1))

        cp_flip = [0]

        def psum_copy(dst, src, allow_act):
            """PSUM->SBUF copy on DVE, or alternating DVE/Act when the
            Activation engine has no exp backlog in this segment."""
            if allow_act and cp_flip[0] % 2 == 1:
                nc.scalar.copy(dst, src)
            else:
                nc.vector.tensor_copy(dst, src)
            cp_flip[0] += 1

        def emit_proj_t(x_tiles, w_sb, dst_sb, t, allow_act):
            """Project one head-pair t: 4 chunks of 512 cols as 2-chunk
            regions of two [128,1024] psums, contraction OUTER so the
            accumulation streams with the x DMAs; then RoPE per chunk."""
            pss = [psum.tile([128, QCW], FP, tag="sc", bufs=3,
                             name=f"pj_t{t}_{i}") for i in range(2)]
            for k in range(KT):
                for sc in range(4):
                    _mm(nc, pss[sc // 2][:, (sc % 2) * 512:(sc % 2 + 1) * 512],
                        w_sb[:, k * F + t * 128: k * F + (t + 1) * 128],
                        x_tiles[k][:, sc * 512:(sc + 1) * 512],
                        start=(k == 0), stop=(k == KT - 1))
            for sc in range(4):
                ps = pss[sc // 2][:, (sc % 2) * 512:(sc % 2 + 1) * 512]
                lo = t * S + sc * 512
                stage = rpool.tile([128, 512], BF, tag="st")
                psum_copy(stage[:], ps, allow_act)
                rot_ps = psum.tile([128, 512], FP, tag="sc", bufs=3, name=f"rot_t{t}_c{sc}")
                _mm(nc, rot_ps[:], prot_sb[:], stage[:], start=True, stop=True)
                rot_sb = rpool.tile([128, 512], BF, tag="rt")
                nc.vector.tensor_mul(rot_sb[:], rot_ps[:],
                                     sin_sb[:, sc * 512:(sc + 1) * 512])
                nc.gpsimd.tensor_mul(dst_sb[:, lo:lo + 512], stage[:],
                                     cos_sb[:, sc * 512:(sc + 1) * 512])
                nc.vector.tensor_add(dst_sb[:, lo:lo + 512],
                                     dst_sb[:, lo:lo + 512], rot_sb[:])

        def emit_v_range(st_lo, st_hi, allow_act):
            for st in range(st_lo, st_hi):
                pv = psum.tile([128, 512], FP, tag="sc", bufs=3, name=f"pv{st}")
                for k in range(KT):
                    _mm(nc, pv[:, 0:256], xv_t[k][:, st * 128:(st + 1) * 128],
                        wv_sb[:, k * F:(k + 1) * F],
                        start=(k == 0), stop=(k == KT - 1))
                c0 = st * HPG * 65
                dstv = v_sb[:, c0:c0 + HPG * 65].rearrange(
                    "p (h c) -> p h c", h=HPG)[:, :, 0:64]
                srcv = pv[:, 0:256].rearrange("p (h c) -> p h c", h=HPG)
                psum_copy(dstv, srcv, allow_act)

        deferred_norm = [None]

        def flush_norm():
            if deferred_norm[0] is not None:
                deferred_norm[0]()
                deferred_norm[0] = None

        def emit_attn_head(qc, h, fillers=()):
            """One head's attention for q-chunk qc. `fillers` are callbacks
            emitted mid-stream (after kt==2) to interleave PE work that has
            no dependency on this head."""
            t, po = h // 2, (h % 2) * 64
            kt_hi = 8 * qc + 8 if causal else ST
            last_b0 = min(kt_hi - 1, 8 * qc + 3) if causal else ST - 1
            ctx_t = psum.tile([128, QCW], FP, tag="ctx", bufs=1, name=f"ctx{qc}_{h}")
            qbase = t * S + qc * QCW

            def emit_pv(e_t, kt, j0):
                vcol = kt * HPG * 65 + h * 65
                for b in range(2):
                    c0, c1 = max(j0, b * 512), (b + 1) * 512
                    if c0 >= c1:
                        continue
                    stop_kt = last_b0 if b == 0 else kt_hi - 1
                    _mm(nc, ctx_t[0:65, c0:c1], v_sb[:, vcol:vcol + 65],
                        e_t[:, c0:c1],
                        start=(kt == 0), stop=(kt == stop_kt))

            pend = []
            for kt in range(kt_hi):
                j0 = max(0, kt * 128 - qc * QCW) if causal else 0
                kcol = t * S + kt * 128
                s_ps = psum.tile([128, QCW], FP, tag="sc", bufs=3, name=f"s{qc}_{h}_{kt}")
                for b in range(2):
                    c0, c1 = max(j0, b * 512), (b + 1) * 512
                    if c0 >= c1:
                        continue
                    _mm(nc, s_ps[:, c0:c1],
                        kt_sb[po:po + 64, kcol:kcol + 128],
                        qt_sb[po:po + 64, qbase + c0:qbase + c1],
                        start=True, stop=True)
                if kt == 1:
                    flush_norm()
                if kt == 2:
                    for fi in fillers:
                        fi()
                if mask_mode == "general":
                    bt = bpool.tile([128, QCW], BF, tag="bt")
                    nc.sync.dma_start(
                        bt[:], biasT[kt * 128:(kt + 1) * 128,
                                     qc * QCW:(qc + 1) * QCW])
                    nc.vector.tensor_add(s_ps[:], s_ps[:], bt[:])
                e_t = epool.tile([128, QCW], BF, tag="e")
                nc.scalar.activation(e_t[:, j0:QCW], s_ps[:, j0:QCW], EXP)
                if causal and kt * 128 >= qc * QCW:
                    # zero the above-diagonal wedge of this k-tile
                    nc.gpsimd.tensor_mul(e_t[:, j0:j0 + 128],
                                         e_t[:, j0:j0 + 128], trimul_sb[:])
                pend.append((e_t, kt, j0))
                if len(pend) > 4:
                    emit_pv(*pend.pop(0))
            for p_ in pend:
                emit_pv(*p_)

            def norm(ctx_t=ctx_t, t=t, po=po, qc=qc):
                # rows 0:64 are ctx^T, row 64 is sum(exp)
                r_sb = npool.tile([1, QCW], FPR, tag="r")
                with nc.allow_low_precision(reason="float32r == fp32 width"):
                    nc.vector.reciprocal(r_sb[:], ctx_t[64:65, :])
                rb_ps = psum.tile([64, QCW], FP, tag="sc", bufs=3, name="rbps")
                for bank in range(2):
                    _mm(nc, rb_ps[:, bank * 512:(bank + 1) * 512], ones_col[:],
                        r_sb[:, bank * 512:(bank + 1) * 512],
                        start=True, stop=True)
                rb_sb = npool.tile([64, QCW], FP, tag="rb")
                nc.vector.tensor_copy(rb_sb[:], rb_ps[:])
                nc.vector.tensor_mul(
                    ctxn_sb[po:po + 64, t * S + qc * QCW: t * S + (qc + 1) * QCW],
                    ctx_t[0:64, :], rb_sb[:])

            deferred_norm[0] = norm

        def emit_oproj_st(st, allow_act, out_eng=None):
            o_ps = psum.tile([128, QCW], FP, tag="sc", bufs=3, name=f"ops{st}")
            for ec in range(2):
                for ft in range(2):
                    _mm(nc, o_ps[:, ec * 512:(ec + 1) * 512],
                        ctxn_sb[:, ft * S + st * 128: ft * S + (st + 1) * 128],
                        wo_sb[:, ft * D + ec * 512: ft * D + (ec + 1) * 512],
                        start=(ft == 0), stop=(ft == 1))
            o_sb = opool.tile([128, QCW], BF, tag="o")
            psum_copy(o_sb[:], o_ps[:], allow_act)
            (out_eng or nc.sync).dma_start(outp[st * 128:(st + 1) * 128, :], o_sb[:])

        # ---------------- emission schedule ----------------
        v_split = 8 if causal else ST

        emit_proj_t(xq_t, wq_sb, qt_sb, 0, allow_act=True)
        emit_proj_t(xk_t, wk_sb, kt_sb, 0, allow_act=True)
        emit_v_range(0, v_split, allow_act=True)
        emit_attn_head(0, 0)
        emit_attn_head(0, 1)
        # t=1 projections run while the h0/h1 exps drain on Act
        emit_attn_head_fillers = [
            lambda: emit_proj_t(xq_t, wq_sb, qt_sb, 1, allow_act=True),
            lambda: emit_proj_t(xk_t, wk_sb, kt_sb, 1, allow_act=True),
        ]
        # emit them immediately after h1's stream (no dependency on h2/h3)
        for fi in emit_attn_head_fillers:
            fi()
        emit_attn_head(0, 2)
        emit_attn_head(0, 3, fillers=(
            (lambda: emit_v_range(v_split, ST, allow_act=False),)
            if v_split < ST else ()))
        # oproj qc0 interleaved into attn qc1 h0 (emitted as filler);
        # norm(h3,qc0) flushes at h(1,0) kt==2, before the filler runs
        # oproj qc0 interleaved into attn qc1 h0 (emitted as filler)
        oproj0 = list(range(0, 8))

        def oproj0_fill():
            for st in oproj0:
                emit_oproj_st(st, allow_act=False)

        emit_attn_head(1, 0, fillers=(oproj0_fill,))
        emit_attn_head(1, 1)
        emit_attn_head(1, 2)
        emit_attn_head(1, 3)
        flush_norm()
        for st in range(8, 16):
            emit_oproj_st(st, allow_act=True,
                          out_eng=nc.sync if st % 2 == 0 else nc.scalar)
    _hoist_waits(nc)
    return nc


def _get_nc(mask_mode):
    if mask_mode not in _nc_cache:
        _nc_cache[mask_mode] = build_nc(mask_mode)
    return _nc_cache[mask_mode]


def _bf16(x):
    import ml_dtypes
    return np.ascontiguousarray(x.astype(ml_dtypes.bfloat16))


def _rope_tables():
    """cos/sin tables in [128, S] layout (64-row block tiled twice). Sign of
    the rotate-half term lives in the P permutation matrix, not in sin."""
    inv_freq = (1.0 / (10000.0 ** (np.arange(0, DK, 2, dtype=np.float32)
                                   / np.float32(DK)))).astype(np.float32)
    t = np.arange(S, dtype=np.float32)
    freqs = np.outer(t, inv_freq).astype(np.float32)      # (S, 32)
    emb = np.concatenate([freqs, freqs], axis=-1)         # (S, 64)
    cos64 = np.cos(emb).T.astype(np.float32)              # (64, S)
    sin64 = np.sin(emb).T.astype(np.float32)
    cos128 = np.ascontiguousarray(np.tile(cos64, (2, 1)))
    sin128 = np.ascontiguousarray(np.tile(sin64, (2, 1)))
    return cos128, sin128


def _prot():
    """P^T for the rotate-half matmul: out = P @ x with
    P[m, m+32] = -1 (m in [0,32)), P[m, m-32] = +1 (m in [32,64)),
    repeated per 64-row head block."""
    P = np.zeros((128, 128), dtype=np.float32)
    for base in (0, 64):
        for m in range(32):
            P[base + m, base + m + 32] = -1.0
            P[base + 32 + m, base + m] = 1.0
    return np.ascontiguousarray(P.T)


def _mask_mode(m2d):
    if (m2d != 0).all():
        return "full"
    if np.array_equal(m2d != 0, np.tril(np.ones((S, S), dtype=bool))):
        return "causal"
    return "general"


def _prepare(inputs):
    q = np.asarray(inputs["query"], dtype=np.float32)
    k = np.asarray(inputs["key"], dtype=np.float32)
    v = np.asarray(inputs["value"], dtype=np.float32)
    mask = np.asarray(inputs["mask"])
    Wq = np.asarray(inputs["W_q"], dtype=np.float32)
    Wk = np.asarray(inputs["W_k"], dtype=np.float32)
    Wv = np.asarray(inputs["W_v"], dtype=np.float32)
    Wo = np.asarray(inputs["W_o"], dtype=np.float32)

    modes = [_mask_mode(mask[b, 0]) for b in range(NB)]
    if all(m == "causal" for m in modes):
        mode = "causal"
    elif all(m == "full" for m in modes):
        mode = "full"
    else:
        mode = "general"
    nc = _get_nc(mode)

    cos128, sin128 = _rope_tables()
    scale = np.float32(1.0 / np.sqrt(DK))
    if mode == "causal":
        kk = np.arange(128)[:, None]
        qq = np.arange(128)[None, :]
        trimulD = (kk <= qq).astype(np.float32)

    xT = {}
    biasTs = {}
    for b in range(NB):
        xT[b] = (_bf16(q[b].T), _bf16(k[b].T), _bf16(v[b].T))
        if mode == "general":
            biasTs[b] = _bf16(np.where(mask[b, 0].T != 0, np.float32(0.0),
                                       np.float32(NEG)).astype(np.float32))

    cosb, sinb, protb = _bf16(cos128), _bf16(sin128), _bf16(_prot())
    in_maps = []
    for core in range(8):
        b, g = divmod(core, 4)
        rows = slice(g * F, (g + 1) * F)
        m = {
            "xqT": xT[b][0], "xkT": xT[b][1], "xvT": xT[b][2],
            "wqT": _bf16((Wq[rows] * scale).T),
            "wkT": _bf16(Wk[rows].T),
            "wvT": _bf16(Wv[rows].T),
            "woT": _bf16(Wo[:, rows].T),
            "cosS": cosb, "sinS": sinb, "protD": protb,
        }
        if mode == "general":
            m["biasT"] = biasTs[b]
        if mode == "causal":
            m["trimulD"] = _bf16(trimulD)
        in_maps.append(m)
    return nc, in_maps


def _gather(res):
    out = np.zeros((NB, S, D), dtype=np.float32)
    for core in range(8):
        out[core // 4] += np.asarray(res.results[core]["outp"],
                                     dtype=np.float32)
    return out


def kernel(**inputs):
    from concourse import bass_utils

    nc, in_maps = _prepare(inputs)
    res = bass_utils.run_bass_kernel_spmd(nc, in_maps, core_ids=list(range(8)))
    return _gather(res)


def run_traced(**inputs):
    """Run once with NTFF tracing; returns (out, exec_time_ns, raw results)."""
    from concourse import bass_utils

    nc, in_maps = _prepare(inputs)
    res = bass_utils.run_bass_kernel_spmd(nc, in_maps, core_ids=list(range(8)),
                                          trace=True)
    return _gather(res), res.exec_time_ns, res


# revision 3
# speedup vs baseline: 1.6829x; 1.6829x over previous
"""Trainium2 Bass kernel: multi-head attention (B=2, S=2048, D=1024, H=16,
d_k=64) with RoPE and masked softmax, sharded over 8 NeuronCores as
(batch x head-group): core = b*4 + g handles batch b, heads [4g, 4g+4).

Host packs ALL per-core inputs into ONE [3968, 2048] bf16 tensor (the axon
exec path costs ~77us/exec per bound tensor): rows [0,3072) are xq/xk/xv in
[d, s] layout; the rest are 128-row blocks pre-packed in SBUF layout (wq,
wk, wv, wo, cos, sin, prot|trimul).

Per-core device program (matmul inputs bf16, PSUM accumulation f32):
  1. All x tiles resident in SBUF (two HWDGE queues + Pool SWDGE, x first,
     weights need-ordered); projections run k-outer into 2-chunk PSUM
     tiles so accumulation streams with the input DMAs.
  2. RoPE rotate-half via a PE permutation matmul (sign folded into a
     128x128 block-diag P), then dst = stage*cos + rot*sin on DVE/Pool.
  3. Causal shrink: scores/exp/PV touch only [j0:] per k-tile; the
     diagonal-block mask is a 0/1 multiply on e_t after exp (SBUF, Pool)
     -- scores stay O(10) so exp never overflows. A ones column per
     65-wide V head block makes the PV matmul emit softmax denominators.
  4. Phase interleave: [Qt0 Kt0][V 0..7][attn qc0 h0 h1][Qt1 Kt1]
     [attn qc0 h2 h3 + V 8..15][oproj qc0 inside attn qc1][attn qc1]
     [oproj qc1]; the Activation engine (exp, the phase-2 pacer) starts
     ~25us in. Head normalization (reciprocal -> K=1 outer-product
     broadcast -> scale) is emitted deferred into the next head's score
     stream so the in-order PE never parks on the DVE reciprocal.
  5. Single PSUM pool: sc 3x[128,1024] + ctx 1x[128,1024] = all 8 banks,
     shared by projection/rot/V/scores/broadcast/output-proj psums.
Output partials are written bf16; the host sums the 4 group partials per
batch in f32. Wait counts >1 are hoisted onto single-wait no-ops after
scheduling (walrus codegen limitation).
"""
import sys

sys.path.insert(0, "/opt/trn_rl_repo")

from contextlib import ExitStack

import numpy as np

import concourse.bass as bass
import concourse.mybir as mybir
import concourse.tile as tile

FP = mybir.dt.float32
FPR = mybir.dt.float32r
BF = mybir.dt.bfloat16
EXP = mybir.ActivationFunctionType.Exp

D = 1024        # d_model
S = 2048        # sequence length
NB = 2          # batches
HPG = 4         # heads per group (= per core)
DK = 64         # head dim
F = HPG * DK    # 256 = group feature width
KT = D // 128   # 8 contraction tiles for projections
ST = S // 128   # 16 seq tiles
QCW = 1024      # q-chunk width (= 2 PSUM banks)
NQC = S // QCW  # 2
NEG = -1e9

_nc_cache = {}


def _mm(nc, out, lhsT, rhs, **kw):
    nc.tensor.matmul(out, lhsT, rhs, **kw)


def _hoist_waits(nc):
    """Several walrus codegen structs (fused-LDW matmul, pseudo direct2d
    DMA, ...) only have room for a single sync wait. Hoist every limited
    instruction's waits (when >1) onto same-engine no-ops inserted just
    before it."""
    f = nc.m.functions[0]

    def engine_builder(eng):
        return {
            mybir.EngineType.PE: nc.tensor,
            mybir.EngineType.DVE: nc.vector,
            mybir.EngineType.Activation: nc.scalar,
            mybir.EngineType.Pool: nc.gpsimd,
            mybir.EngineType.SP: nc.sync,
        }[eng]

    def fresh_nop(eng):
        inst = engine_builder(eng).nop().ins
        for b in f.blocks:
            for i, x in enumerate(b.instructions):
                if x is inst:
                    del b.instructions[i]
                    return inst
        raise RuntimeError("created nop not found in any block")

    total = 0
    for blk in f.blocks:
        out = []
        for inst in blk.instructions:
            si = inst.sync_info
            if si is not None and len(si.on_wait) > 1:
                for w in si.on_wait[:-1]:
                    nop = fresh_nop(inst.engine)
                    nop.sync_info = mybir.SyncInfo(on_wait=[w], on_update=[])
                    out.append(nop)
                    total += 1
                inst.sync_info = mybir.SyncInfo(on_wait=[si.on_wait[-1]],
                                                on_update=list(si.on_update))
            out.append(inst)
        blk.instructions[:] = out
    return total


def build_nc(mask_mode):
    """mask_mode: 'causal' | 'full' | 'general'."""
    assert mask_mode in ("causal", "full", "general")
    nc = bass.Bass("TRN2", target_bir_lowering=False, debug=False, num_devices=8)

    # ONE packed input (the axon exec path costs ~77us per tensor per exec):
    # rows [0,1024) xq^T, [1024,2048) xk^T, [2048,3072) xv^T -- [d, s] layouts;
    # then 128-row blocks pre-packed in SBUF layout: wq, wk, wv, wo, cos, sin,
    # and prot|trimul (cols 0:128 / 128:256).
    xD = nc.dram_tensor("xD", [3968, S], BF, kind="ExternalInput").ap()
    WQR, WKR, WVR, WOR = 3072, 3200, 3328, 3456
    COSR, SINR, MISCR = 3584, 3712, 3840
    if mask_mode == "general":
        biasT = nc.dram_tensor("biasT", [S, S], BF, kind="ExternalInput").ap()
    outp = nc.dram_tensor("outp", [S, D], BF, kind="ExternalOutput").ap()

    causal = mask_mode == "causal"

    with tile.TileContext(nc) as tc, ExitStack() as ctx:
        const = ctx.enter_context(tc.tile_pool(name="const", bufs=1))
        qk = ctx.enter_context(tc.tile_pool(name="qk", bufs=1))
        xres = ctx.enter_context(tc.tile_pool(name="xres", bufs=8))
        rpool = ctx.enter_context(tc.tile_pool(name="rope", bufs=4))
        psum = ctx.enter_context(tc.tile_pool(name="psum", bufs=1, space="PSUM"))
        epool = ctx.enter_context(tc.tile_pool(name="exp", bufs=6))
        npool = ctx.enter_context(tc.tile_pool(name="norm", bufs=2))
        opool = ctx.enter_context(tc.tile_pool(name="ost", bufs=6))
        if mask_mode == "general":
            bpool = ctx.enter_context(tc.tile_pool(name="bias", bufs=2))

        wq_sb = const.tile([128, KT * F], BF)
        wk_sb = const.tile([128, KT * F], BF)
        wv_sb = const.tile([128, KT * F], BF)
        wo_sb = const.tile([128, 2 * D], BF)
        cos_sb = const.tile([128, S], BF)
        sin_sb = const.tile([128, S], BF)
        prot_sb = const.tile([128, 128], BF)
        ones_f32 = const.tile([1, 64], FP)
        nc.vector.memset(ones_f32[:], 1.0)
        ones_col = const.tile([1, 64], FPR)
        nc.vector.tensor_copy(ones_col[:], ones_f32[:])
        ones64 = const.tile([128, 64], BF)
        nc.vector.memset(ones64[:], 1.0)

        # --- DMA issue: everything up-front on both queues, need-ordered.
        # All loads are plain column/row slices of the packed xD (weights are
        # host-packed in their SBUF layouts).
        nc.sync.dma_start(wq_sb[:, 0:F], xD[WQR:WQR + 128, 0:F])
        nc.scalar.dma_start(wq_sb[:, F:4 * F], xD[WQR:WQR + 128, F:4 * F])
        nc.sync.dma_start(wq_sb[:, 4 * F:KT * F], xD[WQR:WQR + 128, 4 * F:KT * F])

        def load_x(row0, label, odd_eng, halves=False):
            tiles = []
            for k in range(KT):
                xt = xres.tile([128, S], BF, tag=f"{label}", name=f"{label}{k}")
                dma_eng = nc.sync if k % 2 == 0 else odd_eng
                r = row0 + k * 128
                if halves:
                    dma_eng.dma_start(xt[:, 0:1024], xD[r:r + 128, 0:1024])
                    dma_eng.dma_start(xt[:, 1024:S], xD[r:r + 128, 1024:S])
                else:
                    dma_eng.dma_start(xt[:], xD[r:r + 128, :])
                tiles.append(xt)
            return tiles

        xq_t = load_x(0, "xq", nc.scalar, halves=True)
        nc.scalar.dma_start(cos_sb[:], xD[COSR:COSR + 128, :])
        nc.scalar.dma_start(sin_sb[:], xD[SINR:SINR + 128, :])
        nc.scalar.dma_start(prot_sb[:], xD[MISCR:MISCR + 128, 0:128])
        if causal:
            trimul_sb = const.tile([128, 128], BF)
            nc.scalar.dma_start(trimul_sb[:], xD[MISCR:MISCR + 128, 128:256])
        nc.gpsimd.dma_start(wk_sb[:], xD[WKR:WKR + 128, :])
        xk_t = load_x(D, "xk", nc.gpsimd)
        nc.sync.dma_start(wv_sb[:], xD[WVR:WVR + 128, :])
        xv_t = load_x(2 * D, "xv", nc.gpsimd)
        nc.sync.dma_start(wo_sb[:], xD[WOR:WOR + 128, :])

        # persistent activations: [p, t*S + s] layouts (t-tile 0: heads 0,1;
        # t-tile 1: heads 2,3 of the group)
        qt_sb = qk.tile([128, 2 * S], BF)
        kt_sb = qk.tile([128, 2 * S], BF)
        # V in [s, f] layout with a ones column per head: 65-wide head blocks
        v_sb = qk.tile([128, ST * HPG * 65], BF)
        ctxn_sb = qk.tile([128, 2 * S], BF)

        ones_ap = v_sb[:].rearrange("p (b c) -> p b c", c=65)[:, :, 64:65]
        nc.vector.tensor_copy(ones_ap, ones64[:].rearrange("p (b o) -> p b o", o=1))

        cp_flip = [0]

        def psum_copy(dst, src, allow_act):
            """PSUM->SBUF copy on DVE, or alternating DVE/Act when the
            Activation engine has no exp backlog in this segment."""
            if allow_act and cp_flip[0] % 2 == 1:
                nc.scalar.copy(dst, src)
            else:
                nc.vector.tensor_copy(dst, src)
            cp_flip[0] += 1

        def emit_proj_t(x_tiles, w_sb, dst_sb, t, allow_act):
            """Project one head-pair t: 4 chunks of 512 cols as 2-chunk
            regions of two [128,1024] psums, contraction OUTER so the
            accumulation streams with the x DMAs; then RoPE per chunk."""
            pss = [psum.tile([128, QCW], FP, tag="sc", bufs=3,
                             name=f"pj_t{t}_{i}") for i in range(2)]
            for k in range(KT):
                for sc in range(4):
                    _mm(nc, pss[sc // 2][:, (sc % 2) * 512:(sc % 2 + 1) * 512],
                        w_sb[:, k * F + t * 128: k * F + (t + 1) * 128],
                        x_tiles[k][:, sc * 512:(sc + 1) * 512],
                        start=(k == 0), stop=(k == KT - 1))
            for sc in range(4):
                ps = pss[sc // 2][:, (sc % 2) * 512:(sc % 2 + 1) * 512]
                lo = t * S + sc * 512
                stage = rpool.tile([128, 512], BF, tag="st")
                psum_copy(stage[:], ps, allow_act)
                rot_ps = psum.tile([128, 512], FP, tag="sc", bufs=3, name=f"rot_t{t}_c{sc}")
                _mm(nc, rot_ps[:], prot_sb[:], stage[:], start=True, stop=True)
                rot_sb = rpool.tile([128, 512], BF, tag="rt")
                nc.vector.tensor_mul(rot_sb[:], rot_ps[:],
                                     sin_sb[:, sc * 512:(sc + 1) * 512])
                nc.gpsimd.tensor_mul(dst_sb[:, lo:lo + 512], stage[:],
                                     cos_sb[:, sc * 512:(sc + 1) * 512])
                nc.vector.tensor_add(dst_sb[:, lo:lo + 512],
                                     dst_sb[:, lo:lo + 512], rot_sb[:])

        def emit_v_range(st_lo, st_hi, allow_act):
            for st in range(st_lo, st_hi):
                pv = psum.tile([128, 512], FP, tag="sc", bufs=3, name=f"pv{st}")
                for k in range(KT):
                    _mm(nc, pv[:, 0:256], xv_t[k][:, st * 128:(st + 1) * 128],
                        wv_sb[:, k * F:(k + 1) * F],
                        start=(k == 0), stop=(k == KT - 1))
                c0 = st * HPG * 65
                dstv = v_sb[:, c0:c0 + HPG * 65].rearrange(
                    "p (h c) -> p h c", h=HPG)[:, :, 0:64]
                srcv = pv[:, 0:256].rearrange("p (h c) -> p h c", h=HPG)
                psum_copy(dstv, srcv, allow_act)

        deferred_norm = [None]

        def flush_norm():
            if deferred_norm[0] is not None:
                deferred_norm[0]()
                deferred_norm[0] = None

        def emit_attn_head(qc, h, fillers=()):
            """One head's attention for q-chunk qc. `fillers` are callbacks
            emitted mid-stream (after kt==2) to interleave PE work that has
            no dependency on this head."""
            t, po = h // 2, (h % 2) * 64
            kt_hi = 8 * qc + 8 if causal else ST
            last_b0 = min(kt_hi - 1, 8 * qc + 3) if causal else ST - 1
            ctx_t = psum.tile([128, QCW], FP, tag="ctx", bufs=1, name=f"ctx{qc}_{h}")
            qbase = t * S + qc * QCW

            def emit_pv(e_t, kt, j0):
                vcol = kt * HPG * 65 + h * 65
                for b in range(2):
                    c0, c1 = max(j0, b * 512), (b + 1) * 512
                    if c0 >= c1:
                        continue
                    stop_kt = last_b0 if b == 0 else kt_hi - 1
                    _mm(nc, ctx_t[0:65, c0:c1], v_sb[:, vcol:vcol + 65],
                        e_t[:, c0:c1],
                        start=(kt == 0), stop=(kt == stop_kt))

            pend = []
            for kt in range(kt_hi):
                j0 = max(0, kt * 128 - qc * QCW) if causal else 0
                kcol = t * S + kt * 128
                s_ps = psum.tile([128, QCW], FP, tag="sc", bufs=3, name=f"s{qc}_{h}_{kt}")
                for b in range(2):
                    c0, c1 = max(j0, b * 512), (b + 1) * 512
                    if c0 >= c1:
                        continue
                    _mm(nc, s_ps[:, c0:c1],
                        kt_sb[po:po + 64, kcol:kcol + 128],
                        qt_sb[po:po + 64, qbase + c0:qbase + c1],
                        start=True, stop=True)
                if kt == 1:
                    flush_norm()
                if kt == 2:
                    for fi in fillers:
                        fi()
                if mask_mode == "general":
                    bt = bpool.tile([128, QCW], BF, tag="bt")
                    nc.sync.dma_start(
                        bt[:], biasT[kt * 128:(kt + 1) * 128,
                                     qc * QCW:(qc + 1) * QCW])
                    nc.vector.tensor_add(s_ps[:], s_ps[:], bt[:])
                e_t = epool.tile([128, QCW], BF, tag="e")
                nc.scalar.activation(e_t[:, j0:QCW], s_ps[:, j0:QCW], EXP)
                if causal and kt * 128 >= qc * QCW:
                    # zero the above-diagonal wedge of this k-tile
                    nc.gpsimd.tensor_mul(e_t[:, j0:j0 + 128],
                                         e_t[:, j0:j0 + 128], trimul_sb[:])
                pend.append((e_t, kt, j0))
                if len(pend) > 4:
                    emit_pv(*pend.pop(0))
            for p_ in pend:
                emit_pv(*p_)

            def norm(ctx_t=ctx_t, t=t, po=po, qc=qc):
                # rows 0:64 are ctx^T, row 64 is sum(exp)
                r_sb = npool.tile([1, QCW], FPR, tag="r")
                with nc.allow_low_precision(reason="float32r == fp32 width"):
                    nc.vector.reciprocal(r_sb[:], ctx_t[64:65, :])
                rb_ps = psum.tile([64, QCW], FP, tag="sc", bufs=3, name="rbps")
                for bank in range(2):
                    _mm(nc, rb_ps[:, bank * 512:(bank + 1) * 512], ones_col[:],
                        r_sb[:, bank * 512:(bank + 1) * 512],
                        start=True, stop=True)
                rb_sb = npool.tile([64, QCW], FP, tag="rb")
                nc.vector.tensor_copy(rb_sb[:], rb_ps[:])
                nc.vector.tensor_mul(
                    ctxn_sb[po:po + 64, t * S + qc * QCW: t * S + (qc + 1) * QCW],
                    ctx_t[0:64, :], rb_sb[:])

            deferred_norm[0] = norm

        def emit_oproj_st(st, allow_act, out_eng=None):
            o_ps = psum.tile([128, QCW], FP, tag="sc", bufs=3, name=f"ops{st}")
            for ec in range(2):
                for ft in range(2):
                    _mm(nc, o_ps[:, ec * 512:(ec + 1) * 512],
                        ctxn_sb[:, ft * S + st * 128: ft * S + (st + 1) * 128],
                        wo_sb[:, ft * D + ec * 512: ft * D + (ec + 1) * 512],
                        start=(ft == 0), stop=(ft == 1))
            o_sb = opool.tile([128, QCW], BF, tag="o")
            psum_copy(o_sb[:], o_ps[:], allow_act)
            (out_eng or nc.sync).dma_start(outp[st * 128:(st + 1) * 128, :], o_sb[:])

        # ---------------- emission schedule ----------------
        v_split = 8 if causal else ST

        emit_proj_t(xq_t, wq_sb, qt_sb, 0, allow_act=True)
        emit_proj_t(xk_t, wk_sb, kt_sb, 0, allow_act=True)
        emit_v_range(0, v_split, allow_act=True)
        emit_attn_head(0, 0)
        emit_attn_head(0, 1)
        # t=1 projections run while the h0/h1 exps drain on Act
        emit_attn_head_fillers = [
            lambda: emit_proj_t(xq_t, wq_sb, qt_sb, 1, allow_act=True),
            lambda: emit_proj_t(xk_t, wk_sb, kt_sb, 1, allow_act=True),
        ]
        # emit them immediately after h1's stream (no dependency on h2/h3)
        for fi in emit_attn_head_fillers:
            fi()
        emit_attn_head(0, 2)
        emit_attn_head(0, 3, fillers=(
            (lambda: emit_v_range(v_split, ST, allow_act=False),)
            if v_split < ST else ()))
        # oproj qc0 interleaved into attn qc1 h0 (emitted as filler);
        # norm(h3,qc0) flushes at h(1,0) kt==2, before the filler runs
        # oproj qc0 interleaved into attn qc1 h0 (emitted as filler)
        oproj0 = list(range(0, 8))

        def oproj0_fill():
            for st in oproj0:
                emit_oproj_st(st, allow_act=False)

        emit_attn_head(1, 0, fillers=(oproj0_fill,))
        emit_attn_head(1, 1)
        emit_attn_head(1, 2)
        emit_attn_head(1, 3)
        flush_norm()
        for st in range(8, 16):
            emit_oproj_st(st, allow_act=True,
                          out_eng=nc.sync if st % 2 == 0 else nc.scalar)
    _hoist_waits(nc)
    return nc


def _get_nc(mask_mode):
    if mask_mode not in _nc_cache:
        _nc_cache[mask_mode] = build_nc(mask_mode)
    return _nc_cache[mask_mode]


def _bf16(x):
    import ml_dtypes
    return np.ascontiguousarray(x.astype(ml_dtypes.bfloat16))


def _rope_tables():
    """cos/sin tables in [128, S] layout (64-row block tiled twice). Sign of
    the rotate-half term lives in the P permutation matrix, not in sin."""
    inv_freq = (1.0 / (10000.0 ** (np.arange(0, DK, 2, dtype=np.float32)
                                   / np.float32(DK)))).astype(np.float32)
    t = np.arange(S, dtype=np.float32)
    freqs = np.outer(t, inv_freq).astype(np.float32)      # (S, 32)
    emb = np.concatenate([freqs, freqs], axis=-1)         # (S, 64)
    cos64 = np.cos(emb).T.astype(np.float32)              # (64, S)
    sin64 = np.sin(emb).T.astype(np.float32)
    cos128 = np.ascontiguousarray(np.tile(cos64, (2, 1)))
    sin128 = np.ascontiguousarray(np.tile(sin64, (2, 1)))
    return cos128, sin128


def _prot():
    """P^T for the rotate-half matmul: out = P @ x with
    P[m, m+32] = -1 (m in [0,32)), P[m, m-32] = +1 (m in [32,64)),
    repeated per 64-row head block."""
    P = np.zeros((128, 128), dtype=np.float32)
    for base in (0, 64):
        for m in range(32):
            P[base + m, base + m + 32] = -1.0
            P[base + 32 + m, base + m] = 1.0
    return np.ascontiguousarray(P.T)


def _mask_mode(m2d):
    if (m2d != 0).all():
        return "full"
    if np.array_equal(m2d != 0, np.tril(np.ones((S, S), dtype=bool))):
        return "causal"
    return "general"


def _prepare(inputs):
    q = np.asarray(inputs["query"], dtype=np.float32)
    k = np.asarray(inputs["key"], dtype=np.float32)
    v = np.asarray(inputs["value"], dtype=np.float32)
    mask = np.asarray(inputs["mask"])
    Wq = np.asarray(inputs["W_q"], dtype=np.float32)
    Wk = np.asarray(inputs["W_k"], dtype=np.float32)
    Wv = np.asarray(inputs["W_v"], dtype=np.float32)
    Wo = np.asarray(inputs["W_o"], dtype=np.float32)

    modes = [_mask_mode(mask[b, 0]) for b in range(NB)]
    if all(m == "causal" for m in modes):
        mode = "causal"
    elif all(m == "full" for m in modes):
        mode = "full"
    else:
        mode = "general"
    nc = _get_nc(mode)

    cos128, sin128 = _rope_tables()
    scale = np.float32(1.0 / np.sqrt(DK))

    def sbuf_pack(wT):
        # [D, F] (d-major) -> [128, KT*F] SBUF layout: [p, k*F+f] = wT[k*128+p, f]
        return wT.reshape(KT, 128, F).transpose(1, 0, 2).reshape(128, KT * F)

    xT = {}
    biasTs = {}
    for b in range(NB):
        xT[b] = np.concatenate([q[b].T, k[b].T, v[b].T], axis=0)  # (3072, S) f32
        if mode == "general":
            biasTs[b] = _bf16(np.where(mask[b, 0].T != 0, np.float32(0.0),
                                       np.float32(NEG)).astype(np.float32))

    misc = np.zeros((128, S), dtype=np.float32)
    misc[:, 0:128] = _prot()
    if mode == "causal":
        kk = np.arange(128)[:, None]
        qq = np.arange(128)[None, :]
        misc[:, 128:256] = (kk <= qq).astype(np.float32)

    in_maps = []
    for core in range(8):
        b, g = divmod(core, 4)
        rows = slice(g * F, (g + 1) * F)
        # wo SBUF layout: [p, t*D + e] = woT[t*128 + p, e]
        woT = Wo[:, rows].T  # (F=256, D)
        wo_pack = woT.reshape(2, 128, D).transpose(1, 0, 2).reshape(128, 2 * D)
        blocks = np.concatenate([
            xT[b],
            sbuf_pack((Wq[rows] * scale).T),
            sbuf_pack(Wk[rows].T),
            sbuf_pack(Wv[rows].T),
            wo_pack,
            cos128, sin128, misc,
        ], axis=0)  # (3968, S) f32
        m = {"xD": _bf16(blocks)}
        if mode == "general":
            m["biasT"] = biasTs[b]
        in_maps.append(m)
    return nc, in_maps


def _gather(res):
    out = np.zeros((NB, S, D), dtype=np.float32)
    for core in range(8):
        out[core // 4] += np.asarray(res.results[core]["outp"],
                                     dtype=np.float32)
    return out


def kernel(**inputs):
    from concourse import bass_utils

    nc, in_maps = _prepare(inputs)
    res = bass_utils.run_bass_kernel_spmd(nc, in_maps, core_ids=list(range(8)))
    return _gather(res)


def run_traced(**inputs):
    """Run once with NTFF tracing; returns (out, exec_time_ns, raw results)."""
    from concourse import bass_utils

    nc, in_maps = _prepare(inputs)
    res = bass_utils.run_bass_kernel_spmd(nc, in_maps, core_ids=list(range(8)),
                                          trace=True)
    return _gather(res), res.exec_time_ns, res
